# revision 27
# baseline (speedup 1.0000x reference)
"""BinaryDense kernel for Trainium2: out = sign(x) @ sign(w).

x: [8192, 2048] f32, w: [2048, 2048] f32 -> out: [8192, 2048] f32.

Strategy: data-parallel shard of the batch dim across 8 NeuronCores
(1024 rows each, w replicated). The host ships only the HIGH BYTE of
each f32 (a pure byte-plane slice -- sign bit + 7 exponent bits, which
fully determines the sign), so input DMA is 6MB/core (x 2MB + w 4MB).
Host pre-tiles layouts so each 256KB DMA granule moves >=512B
contiguous lines per partition:
  x_dram [128p, 16ks, 1024m]    (d = ks*128 + p)
  w_dram [128p, 4q, 16ks, 512u] (u = q*512 + u')
Per core:
  - 11 dummy matmuls on scratch data at kernel start warm the PE HAM
    clock gate during the input-DMA head (otherwise the first ~5us of
    real matmuls run at 1.2GHz instead of 2.4GHz).
  - binarize from the u8 high byte ((v < 128) <=> x >= 0), split across
    both elementwise engines at per-output-column granularity (the
    per-j evict scale absorbs the class difference):
      all w, and x columns j3/j6/j7 -> +-0.5 fp8e4 on DVE
        ((v < 128) - 0.5)
      x columns j0-2/j4-5          -> +-1  fp8e4 on ACT
        (Sign(127.5 - v))
  - fp8 DoubleRow matmuls (K=256/instr, N=512): stationary = x slice,
    moving = w granule. Pass 0 runs h-major rows over m-group 0 then
    m-group 1 (8 PSUM banks = 8 concurrent chains), woven with the
    arrival stream; passes 1-3 run 4-wide j-groups with w prefetched a
    pass ahead and evicts of the previous group on the first rows.
  - outputs in ost[j] fp16 (integers <= 2048, exact); stores stream on
    the sync ring behind the inputs; the final group's evicts alternate
    ACT/DVE and its stores alternate across both HWDGE rings to
    shorten the tail.

All arithmetic exact; host fp16->f32 widening exact.
"""

import sys

if "/opt/trn_rl_repo" not in sys.path:
    sys.path.insert(0, "/opt/trn_rl_repo")

import numpy as np

B_FULL, D_IN, UNITS = 8192, 2048, 2048
N_CORES = 8
B_CORE = B_FULL // N_CORES  # 1024
P = 128
KT = D_IN // P              # 16 k-subtiles
NQ = 4                      # n passes of 512 columns
NH = 8                      # DR h-steps per pass (2 k-subtiles each)
MT = B_CORE // P            # 8 m-tiles
N_DUMMY = 9                # PE warm-up matmuls


def build_kernel():
    from concourse import bacc
    import concourse.mybir as mybir
    import concourse.tile as tile

    f32 = mybir.dt.float32
    f16 = mybir.dt.float16
    f8 = mybir.dt.float8e4
    u8 = mybir.dt.uint8

    LT = mybir.AluOpType.is_lt
    SUB = mybir.AluOpType.subtract
    COPY = mybir.ActivationFunctionType.Copy
    SIGN = mybir.ActivationFunctionType.Sign
    DR = mybir.MatmulPerfMode.DoubleRow

    # per-j evict scale: chain sum = alpha * out, w always +-0.5 (DVE)
    #   x on ACT (+-1) -> alpha 0.5 -> scale 2 : j0-2, j4-5
    #   x on DVE (+-0.5) -> alpha 0.25 -> scale 4 : j3, j6-7
    EV_SCALE_J = (2.0, 2.0, 2.0, 4.0, 2.0, 2.0, 4.0, 4.0)

    nc = bacc.Bacc("TRN2", target_bir_lowering=False)
    # const AP for the Sign-activation bias (127.5)
    _bt = nc.alloc_sbuf_tensor("const-f32-127p5", [P, 1], f32)
    nc.gpsimd.memset(_bt.ap(), 127.5)
    nc.const_aps.aps[(f32, 127.5)] = _bt.ap()
    nc.all_engine_barrier()
    x_d = nc.dram_tensor("xhi", [P, KT, B_CORE], u8, kind="ExternalInput")
    w_d = nc.dram_tensor("whi", [P, NQ, KT, 512], u8, kind="ExternalInput")
    o_d = nc.dram_tensor("out", [B_CORE, UNITS], f16, kind="ExternalOutput")

    o_ap = o_d[:].rearrange("(j p) u -> j p u", p=P)  # [MT, 128, U]

    with tile.TileContext(nc) as tc, \
         tc.tile_pool(name="wstage", bufs=8) as wstage, \
         tc.tile_pool(name="xstage", bufs=8) as xstage, \
         tc.tile_pool(name="resident", bufs=1) as resident, \
         tc.tile_pool(name="mpsum", bufs=8, space="PSUM") as mpsum:

        # resident binarized tiles, one per DMA/binarize granule
        # x8[t][g]: ksubs 4t..4t+3, m-cols 512g..512g+511
        x8 = [[resident.tile([P, 4, 512], f8, name=f"x8_{t}_{g}")
               for g in range(2)] for t in range(4)]
        # w8[q][hp]: pass q, ksubs 4hp..4hp+3
        w8 = [[resident.tile([P, 4, 512], f8, name=f"w8_{q}_{hp}")
               for hp in range(4)] for q in range(NQ)]
        ost = [resident.tile([P, UNITS], f16, name=f"ost_{j}")
               for j in range(MT)]

        # --- PE warm-up: dummy matmuls on scratch data ---
        scratch = resident.tile([P, 2, 512], f8, name="warm_scratch")
        nc.gpsimd.memset(scratch, 0)
        ps_warm = mpsum.tile([P, 512], f32, tag="ps", name="ps_warm")
        for _ in range(N_DUMMY):
            nc.tensor.matmul(ps_warm, lhsT=scratch[:, :, 0:P], rhs=scratch,
                             start=True, stop=True, perf_mode=DR)

        def dma_x(t, g):
            xs = xstage.tile([P, 4, 512], u8, tag="xs", name=f"xs_{t}_{g}")
            nc.sync.dma_start(xs, x_d[:][:, 4 * t:4 * t + 4,
                                         512 * g:512 * (g + 1)])
            return xs

        def dma_w(q, hp):
            ws = wstage.tile([P, 4, 512], u8, tag="ws", name=f"ws_{q}_{hp}")
            nc.sync.dma_start(ws, w_d[:][:, q, 4 * hp:4 * hp + 4, :])
            return ws

        def bin_x(t, g, xs):
            # per-j engine classes (evict scale absorbs the factor):
            #   g0: j0-2 ACT Sign +-1, j3 DVE +-0.5
            #   g1: j4-5 ACT Sign +-1, j6-7 DVE +-0.5
            cut = 384 if g == 0 else 256
            dst, s = x8[t][g], xs
            if t == 0 and g == 0:
                # halved ops so row 0's operands land earliest
                for k in (0, 2):
                    nc.scalar.activation(dst[:, k:k + 2, 0:cut],
                                         s[:, k:k + 2, 0:cut], SIGN,
                                         bias=127.5, scale=-1.0)
                    nc.vector.tensor_scalar(dst[:, k:k + 2, cut:512],
                                            s[:, k:k + 2, cut:512],
                                            128.0, 0.5, LT, SUB)
            else:
                nc.scalar.activation(dst[:, :, 0:cut], s[:, :, 0:cut],
                                     SIGN, bias=127.5, scale=-1.0)
                nc.vector.tensor_scalar(dst[:, :, cut:512],
                                        s[:, :, cut:512], 128.0, 0.5,
                                        LT, SUB)

        def bin_w(q, hp, ws):
            if q == 0 and hp == 0:
                for k in (0, 2):
                    nc.vector.tensor_scalar(w8[q][hp][:, k:k + 2, :],
                                            ws[:, k:k + 2, :],
                                            128.0, 0.5, LT, SUB)
            else:
                nc.vector.tensor_scalar(w8[q][hp], ws, 128.0, 0.5, LT, SUB)

        psum_tiles = {}

        def mm(q, j, h):
            g = j // 4
            if (q, j) not in psum_tiles:
                psum_tiles[(q, j)] = mpsum.tile([P, 512], f32, tag="ps",
                                                name=f"ps_{q}_{j}")
            jo = (j % 4) * P
            c = 2 * (h % 2)
            nc.tensor.matmul(
                psum_tiles[(q, j)],
                lhsT=x8[h // 2][g][:, c:c + 2, jo:jo + P],
                rhs=w8[q][h // 2][:, c:c + 2, :],
                start=(h == 0), stop=(h == NH - 1),
                perf_mode=DR,
            )

        def evict(q, j, eng="act"):
            ps = psum_tiles.pop((q, j))
            dst = ost[j][:, 512 * q:512 * (q + 1)]
            sc = EV_SCALE_J[j]
            if eng == "act":
                nc.scalar.activation(dst, ps, COPY, scale=sc)
            else:
                nc.vector.tensor_scalar_mul(dst, ps, sc)

        def store(j, q0, nq, ring=None):
            n0, n1 = 512 * q0, 512 * (q0 + nq)
            (ring or nc.sync).dma_start(o_ap[j, :, n0:n1], ost[j][:, n0:n1])

        # ---------------- emission weave ----------------
        # Pass 0 m-group 0 (j0-3). The very first granule pair is split
        # in half across BOTH HWDGE rings (sync + scalar) so row 0's
        # operand receipts land in parallel ~1us earlier.
        for t in range(4):
            if t == 0:
                wsa = wstage.tile([P, 2, 512], u8, tag="ws", name="ws_0a")
                nc.sync.dma_start(wsa, w_d[:][:, 0, 0:2, :])
                xsa = xstage.tile([P, 2, 512], u8, tag="xs", name="xs_0a")
                nc.sync.dma_start(xsa, x_d[:][:, 0:2, 0:512])
                wsb = wstage.tile([P, 2, 512], u8, tag="ws", name="ws_0b")
                nc.scalar.dma_start(wsb, w_d[:][:, 0, 2:4, :])
                xsb = xstage.tile([P, 2, 512], u8, tag="xs", name="xs_0b")
                nc.scalar.dma_start(xsb, x_d[:][:, 2:4, 0:512])
                for k, xs_, ws_ in ((0, xsa, wsa), (2, xsb, wsb)):
                    nc.vector.tensor_scalar(w8[0][0][:, k:k + 2, :],
                                            ws_[:, 0:2, :], 128.0, 0.5,
                                            LT, SUB)
                    if k == 0:
                        # j0 slice first: unblocks the first real matmul
                        nc.scalar.activation(x8[0][0][:, 0:2, 0:128],
                                             xs_[:, 0:2, 0:128], SIGN,
                                             bias=127.5, scale=-1.0)
                        nc.scalar.activation(x8[0][0][:, 0:2, 128:384],
                                             xs_[:, 0:2, 128:384], SIGN,
                                             bias=127.5, scale=-1.0)
                    else:
                        nc.scalar.activation(x8[0][0][:, k:k + 2, 0:384],
                                             xs_[:, 0:2, 0:384], SIGN,
                                             bias=127.5, scale=-1.0)
                    nc.vector.tensor_scalar(x8[0][0][:, k:k + 2, 384:512],
                                            xs_[:, 0:2, 384:512],
                                            128.0, 0.5, LT, SUB)
            else:
                xs = dma_x(t, 0)
                ws = dma_w(0, t)
                bin_x(t, 0, xs)
                bin_w(0, t, ws)
            for h in (2 * t, 2 * t + 1):
                for j in range(4):
                    mm(0, j, h)

        # Pass 0 m-group 1 (j4-7): x g1 granules + w q1 prefetch; evicts
        # of group 0 interleave at the end of the window. Its h=7 row is
        # deferred: it fuses with the next group's h=0 row below.
        for t in range(4):
            xs = dma_x(t, 1)
            bin_x(t, 1, xs)
            ws = dma_w(1, t)
            bin_w(1, t, ws)
            for h in (2 * t, 2 * t + 1):
                if h == NH - 1:
                    continue  # deferred into the boundary fusion
                for j in range(4, MT):
                    mm(0, j, h)
            if t >= 2:
                evict(0, 2 * (t - 2), "dve")
                evict(0, 2 * (t - 2) + 1, "act")

        # passes 1..3: 4-wide groups; each group's h=0 row is fused
        # j-by-j with the previous group's deferred h=7 row, hiding the
        # chain start/stop pipeline bubble at every group boundary.
        pending_h7 = [(0, j) for j in range(4, MT)]
        for q in range(1, NQ):
            for g in range(2):
                js = list(range(4 * g, 4 * g + 4))
                for (pq, pj), j in zip(pending_h7, js):
                    mm(pq, pj, NH - 1)
                    mm(q, j, 0)
                pend = [(qq, jj) for (qq, jj) in psum_tiles
                        if (qq, jj // 4) != (q, g)]
                ei = 0
                for h in range(1, NH - 1):
                    if g == 0 and q + 1 < NQ and h <= 4:
                        ws = dma_w(q + 1, h - 1)
                        bin_w(q + 1, h - 1, ws)
                    for j in js:
                        mm(q, j, h)
                    if h < 5 and ei < len(pend):
                        evict(*pend[ei], "act" if ei % 2 == 0 else "dve")
                        ei += 1
                for tpl in pend[ei:]:
                    evict(*tpl, "act")
                pending_h7 = [(q, j) for j in js]

        # flush the final group's h=7 row, then tail evicts alternating
        # engines
        for (pq, pj) in pending_h7:
            mm(pq, pj, NH - 1)
        last = sorted(psum_tiles.keys(), key=lambda kv: kv[1])
        for idx, (qq, jj) in enumerate(last):
            evict(qq, jj, "act" if idx % 2 == 0 else "dve")

        # stores: sync ring in availability order; final group full
        # 128KB stores alternating across both rings
        for j in range(MT):
            store(j, 0, 2)
        for j in range(MT):
            store(j, 2, 1)
        for j in range(4):
            store(j, 3, 1)
        for j in range(4, MT):
            store(j, 3, 1, ring=nc.scalar if j % 2 else nc.sync)

    nc.compile()
    return nc


_NC_CACHE = {}
LAST_RESULTS = {}


def _get_nc():
    if "nc" not in _NC_CACHE:
        _NC_CACHE["nc"] = build_kernel()
    return _NC_CACHE["nc"]


def _prep_inputs(x, w):
    """Host-side formatting only: byte-plane slice + retile (no math)."""
    # high byte of each little-endian f32 = sign bit + exp[7:1]
    x_hi = x.view(np.uint8).reshape(B_FULL, D_IN, 4)[:, :, 3]
    w_hi = w.view(np.uint8).reshape(D_IN, UNITS, 4)[:, :, 3]
    # w: [d, u] -> [p, q, s, u']  with d = s*128 + p, u = q*512 + u'
    wt = w_hi.reshape(KT, P, NQ, 512).transpose(1, 2, 0, 3)
    w_core = np.ascontiguousarray(wt)
    in_maps = []
    for c in range(N_CORES):
        shard = x_hi[c * B_CORE:(c + 1) * B_CORE]          # [m, d]
        t = shard.T.reshape(KT, P, B_CORE).transpose(1, 0, 2)
        in_maps.append({
            "xhi": np.ascontiguousarray(t),                # [128,16,1024]
            "whi": w_core,
        })
    return in_maps


def kernel(x, w, _trace=False, _trace_cores=None):
    from concourse.bass_utils import run_bass_kernel_spmd

    x = np.asarray(x, dtype=np.float32)
    w = np.asarray(w, dtype=np.float32)
    assert x.shape == (B_FULL, D_IN) and w.shape == (D_IN, UNITS)

    nc = _get_nc()
    in_maps = _prep_inputs(x, w)
    br = run_bass_kernel_spmd(
        nc, in_maps, list(range(N_CORES)),
        trace=_trace, trace_cores=_trace_cores,
    )
    LAST_RESULTS["br"] = br
    out = np.concatenate(
        [br.results[c]["out"].astype(np.float32) for c in range(N_CORES)],
        axis=0,
    )
    return out


if __name__ == "__main__":
    rng = np.random.default_rng(0)
    x = rng.standard_normal((B_FULL, D_IN), dtype=np.float32)
    w = (rng.standard_normal((D_IN, UNITS), dtype=np.float32) * 0.1).astype(
        np.float32
    )
    out = kernel(x, w)
    exp = np.sign(x + (x == 0)) @ np.sign(w + (w == 0))
    print("max abs err:", np.max(np.abs(out - exp)))


# revision 28
# speedup vs baseline: 1.0790x; 1.0790x over previous
"""BinaryDense kernel for Trainium2: out = sign(x) @ sign(w).

x: [8192, 2048] f32, w: [2048, 2048] f32 -> out: [8192, 2048] f32.

Strategy: data-parallel shard of the batch dim across 8 NeuronCores
(1024 rows each, w replicated). The host ships only the HIGH BYTE of
each f32 (a pure byte-plane slice -- sign bit + 7 exponent bits, which
fully determines the sign), so input DMA is 6MB/core (x 2MB + w 4MB).
Host pre-tiles layouts so each 256KB DMA granule moves >=512B
contiguous lines per partition:
  x_dram [128p, 16ks, 1024m]    (d = ks*128 + p)
  w_dram [128p, 4q, 16ks, 512u] (u = q*512 + u')
Per core:
  - 11 dummy matmuls on scratch data at kernel start warm the PE HAM
    clock gate during the input-DMA head (otherwise the first ~5us of
    real matmuls run at 1.2GHz instead of 2.4GHz).
  - binarize from the u8 high byte ((v < 128) <=> x >= 0), split across
    both elementwise engines at per-output-column granularity (the
    per-j evict scale absorbs the class difference):
      all w, and x columns j3/j6/j7 -> +-0.5 fp8e4 on DVE
        ((v < 128) - 0.5)
      x columns j0-2/j4-5          -> +-1  fp8e4 on ACT
        (Sign(127.5 - v))
  - fp8 DoubleRow matmuls (K=256/instr, N=512): stationary = x slice,
    moving = w granule. Pass 0 runs h-major rows over m-group 0 then
    m-group 1 (8 PSUM banks = 8 concurrent chains), woven with the
    arrival stream; passes 1-3 run 4-wide j-groups with w prefetched a
    pass ahead and evicts of the previous group on the first rows.
  - outputs in ost[j] fp16 (integers <= 2048, exact); stores stream on
    the sync ring behind the inputs; the final group's evicts alternate
    ACT/DVE and its stores alternate across both HWDGE rings to
    shorten the tail.

All arithmetic exact; host fp16->f32 widening exact.
"""

import sys

if "/opt/trn_rl_repo" not in sys.path:
    sys.path.insert(0, "/opt/trn_rl_repo")

import numpy as np

B_FULL, D_IN, UNITS = 8192, 2048, 2048
N_CORES = 8
B_CORE = B_FULL // N_CORES  # 1024
P = 128
KT = D_IN // P              # 16 k-subtiles
NQ = 4                      # n passes of 512 columns
NH = 8                      # DR h-steps per pass (2 k-subtiles each)
MT = B_CORE // P            # 8 m-tiles
N_DUMMY = 9                # PE warm-up matmuls


def build_kernel():
    from concourse import bacc
    import concourse.mybir as mybir
    import concourse.tile as tile

    f32 = mybir.dt.float32
    f16 = mybir.dt.float16
    f8 = mybir.dt.float8e4
    u8 = mybir.dt.uint8

    LT = mybir.AluOpType.is_lt
    SUB = mybir.AluOpType.subtract
    COPY = mybir.ActivationFunctionType.Copy
    SIGN = mybir.ActivationFunctionType.Sign
    DR = mybir.MatmulPerfMode.DoubleRow

    # per-j evict scale: chain sum = alpha * out, w always +-0.5 (DVE)
    #   x on ACT (+-1) -> alpha 0.5 -> scale 2 : j0-2, j4-5
    #   x on DVE (+-0.5) -> alpha 0.25 -> scale 4 : j3, j6-7
    EV_SCALE_J = (2.0, 2.0, 2.0, 4.0, 2.0, 2.0, 4.0, 4.0)

    nc = bacc.Bacc("TRN2", target_bir_lowering=False)
    # const AP for the Sign-activation bias (127.5)
    _bt = nc.alloc_sbuf_tensor("const-f32-127p5", [P, 1], f32)
    nc.gpsimd.memset(_bt.ap(), 127.5)
    nc.const_aps.aps[(f32, 127.5)] = _bt.ap()
    nc.all_engine_barrier()
    x_d = nc.dram_tensor("xhi", [P, KT, B_CORE], u8, kind="ExternalInput")
    w_d = nc.dram_tensor("whi", [P, NQ, KT, 512], u8, kind="ExternalInput")
    o_d = nc.dram_tensor("out", [B_CORE, UNITS], f16, kind="ExternalOutput")

    o_ap = o_d[:].rearrange("(j p) u -> j p u", p=P)  # [MT, 128, U]

    with tile.TileContext(nc) as tc, \
         tc.tile_pool(name="wstage", bufs=8) as wstage, \
         tc.tile_pool(name="xstage", bufs=8) as xstage, \
         tc.tile_pool(name="resident", bufs=1) as resident, \
         tc.tile_pool(name="mpsum", bufs=8, space="PSUM") as mpsum:

        # resident binarized tiles, one per DMA/binarize granule
        # x8[t][g]: ksubs 4t..4t+3, m-cols 512g..512g+511
        x8 = [[resident.tile([P, 4, 512], f8, name=f"x8_{t}_{g}")
               for g in range(2)] for t in range(4)]
        # w8[q][hp]: pass q, ksubs 4hp..4hp+3
        w8 = [[resident.tile([P, 4, 512], f8, name=f"w8_{q}_{hp}")
               for hp in range(4)] for q in range(NQ)]
        ost = [resident.tile([P, UNITS], f16, name=f"ost_{j}")
               for j in range(MT)]

        # --- PE warm-up: dummy matmuls on scratch data ---
        scratch = resident.tile([P, 2, 512], f8, name="warm_scratch")
        nc.gpsimd.memset(scratch, 0)
        ps_warm = mpsum.tile([P, 512], f32, tag="ps", name="ps_warm")
        for _ in range(N_DUMMY):
            nc.tensor.matmul(ps_warm, lhsT=scratch[:, :, 0:P], rhs=scratch,
                             start=True, stop=True, perf_mode=DR)

        def dma_x(t, g):
            xs = xstage.tile([P, 4, 512], u8, tag="xs", name=f"xs_{t}_{g}")
            nc.sync.dma_start(xs, x_d[:][:, 4 * t:4 * t + 4,
                                         512 * g:512 * (g + 1)])
            return xs

        def dma_w(q, hp):
            ws = wstage.tile([P, 4, 512], u8, tag="ws", name=f"ws_{q}_{hp}")
            nc.sync.dma_start(ws, w_d[:][:, q, 4 * hp:4 * hp + 4, :])
            return ws

        def bin_x(t, g, xs):
            # per-j engine classes (evict scale absorbs the factor):
            #   g0: j0-2 ACT Sign +-1, j3 DVE +-0.5
            #   g1: j4-5 ACT Sign +-1, j6-7 DVE +-0.5
            cut = 384 if g == 0 else 256
            dst, s = x8[t][g], xs
            if t == 0 and g == 0:
                # halved ops so row 0's operands land earliest
                for k in (0, 2):
                    nc.scalar.activation(dst[:, k:k + 2, 0:cut],
                                         s[:, k:k + 2, 0:cut], SIGN,
                                         bias=127.5, scale=-1.0)
                    nc.vector.tensor_scalar(dst[:, k:k + 2, cut:512],
                                            s[:, k:k + 2, cut:512],
                                            128.0, 0.5, LT, SUB)
            else:
                nc.scalar.activation(dst[:, :, 0:cut], s[:, :, 0:cut],
                                     SIGN, bias=127.5, scale=-1.0)
                nc.vector.tensor_scalar(dst[:, :, cut:512],
                                        s[:, :, cut:512], 128.0, 0.5,
                                        LT, SUB)

        def bin_w(q, hp, ws):
            if q == 0 and hp == 0:
                for k in (0, 2):
                    nc.vector.tensor_scalar(w8[q][hp][:, k:k + 2, :],
                                            ws[:, k:k + 2, :],
                                            128.0, 0.5, LT, SUB)
            else:
                nc.vector.tensor_scalar(w8[q][hp], ws, 128.0, 0.5, LT, SUB)

        psum_tiles = {}

        def mm(q, j, h):
            g = j // 4
            if (q, j) not in psum_tiles:
                psum_tiles[(q, j)] = mpsum.tile([P, 512], f32, tag="ps",
                                                name=f"ps_{q}_{j}")
            jo = (j % 4) * P
            c = 2 * (h % 2)
            nc.tensor.matmul(
                psum_tiles[(q, j)],
                lhsT=x8[h // 2][g][:, c:c + 2, jo:jo + P],
                rhs=w8[q][h // 2][:, c:c + 2, :],
                start=(h == 0), stop=(h == NH - 1),
                perf_mode=DR,
            )

        def evict(q, j, eng="act"):
            ps = psum_tiles.pop((q, j))
            dst = ost[j][:, 512 * q:512 * (q + 1)]
            sc = EV_SCALE_J[j]
            if eng == "act":
                nc.scalar.activation(dst, ps, COPY, scale=sc)
            else:
                nc.vector.tensor_scalar_mul(dst, ps, sc)

        def store(j, q0, nq, ring=None):
            n0, n1 = 512 * q0, 512 * (q0 + nq)
            (ring or nc.sync).dma_start(o_ap[j, :, n0:n1], ost[j][:, n0:n1])

        # ---------------- emission weave ----------------
        # Pass 0 m-group 0 (j0-3). The very first granule pair is split
        # in half across BOTH HWDGE rings (sync + scalar) so row 0's
        # operand receipts land in parallel ~1us earlier.
        for t in range(4):
            if t == 0:
                xsa = xstage.tile([P, 2, 512], u8, tag="xs", name="xs_0a")
                nc.sync.dma_start(xsa, x_d[:][:, 0:2, 0:512])
                xsb = xstage.tile([P, 2, 512], u8, tag="xs", name="xs_0b")
                nc.scalar.dma_start(xsb, x_d[:][:, 2:4, 0:512])
                wsa = wstage.tile([P, 2, 512], u8, tag="ws", name="ws_0a")
                nc.sync.dma_start(wsa, w_d[:][:, 0, 0:2, :])
                wsb = wstage.tile([P, 2, 512], u8, tag="ws", name="ws_0b")
                nc.scalar.dma_start(wsb, w_d[:][:, 0, 2:4, :])
                for k, xs_, ws_ in ((0, xsa, wsa), (2, xsb, wsb)):
                    nc.vector.tensor_scalar(w8[0][0][:, k:k + 2, :],
                                            ws_[:, 0:2, :], 128.0, 0.5,
                                            LT, SUB)
                    nc.scalar.activation(x8[0][0][:, k:k + 2, 0:384],
                                         xs_[:, 0:2, 0:384], SIGN,
                                         bias=127.5, scale=-1.0)
                    nc.vector.tensor_scalar(x8[0][0][:, k:k + 2, 384:512],
                                            xs_[:, 0:2, 384:512],
                                            128.0, 0.5, LT, SUB)
            else:
                xs = dma_x(t, 0)
                ws = dma_w(0, t)
                bin_x(t, 0, xs)
                bin_w(0, t, ws)
            for h in (2 * t, 2 * t + 1):
                for j in range(4):
                    mm(0, j, h)

        # Pass 0 m-group 1 (j4-7): x g1 granules + w q1 prefetch; evicts
        # of group 0 interleave at the end of the window. Its h=7 row is
        # deferred: it fuses with the next group's h=0 row below.
        for t in range(4):
            xs = dma_x(t, 1)
            bin_x(t, 1, xs)
            ws = dma_w(1, t)
            bin_w(1, t, ws)
            for h in (2 * t, 2 * t + 1):
                if h == NH - 1:
                    continue  # deferred into the boundary fusion
                for j in range(4, MT):
                    mm(0, j, h)
            if t >= 2:
                evict(0, 2 * (t - 2), "dve")
                evict(0, 2 * (t - 2) + 1, "act")

        # passes 1..3: 4-wide groups; each group's h=0 row is fused
        # j-by-j with the previous group's deferred h=7 row, hiding the
        # chain start/stop pipeline bubble at every group boundary.
        pending_h7 = [(0, j) for j in range(4, MT)]
        for q in range(1, NQ):
            for g in range(2):
                js = list(range(4 * g, 4 * g + 4))
                for (pq, pj), j in zip(pending_h7, js):
                    mm(pq, pj, NH - 1)
                    mm(q, j, 0)
                pend = [(qq, jj) for (qq, jj) in psum_tiles
                        if (qq, jj // 4) != (q, g)]
                ei = 0
                for h in range(1, NH - 1):
                    if g == 0 and q + 1 < NQ and h <= 4:
                        ws = dma_w(q + 1, h - 1)
                        bin_w(q + 1, h - 1, ws)
                    for j in js:
                        mm(q, j, h)
                    if h < 5 and ei < len(pend):
                        evict(*pend[ei], "act")
                        ei += 1
                for tpl in pend[ei:]:
                    evict(*tpl, "act")
                pending_h7 = [(q, j) for j in js]

        # flush the final group's h=7 row, then tail evicts alternating
        # engines
        for (pq, pj) in pending_h7:
            mm(pq, pj, NH - 1)
        last = sorted(psum_tiles.keys(), key=lambda kv: kv[1])
        for idx, (qq, jj) in enumerate(last):
            evict(qq, jj, "act" if idx % 2 == 0 else "dve")

        # stores: sync ring in availability order; final group full
        # 128KB stores alternating across both rings
        for j in range(MT):
            store(j, 0, 2)
        for j in range(MT):
            store(j, 2, 1)
        for j in range(4):
            store(j, 3, 1)
        for j in range(4, MT):
            store(j, 3, 1, ring=nc.scalar if j % 2 else nc.sync)

    nc.compile()
    return nc


_NC_CACHE = {}
LAST_RESULTS = {}


def _get_nc():
    if "nc" not in _NC_CACHE:
        _NC_CACHE["nc"] = build_kernel()
    return _NC_CACHE["nc"]


def _prep_inputs(x, w):
    """Host-side formatting only: byte-plane slice + retile (no math)."""
    # high byte of each little-endian f32 = sign bit + exp[7:1]
    x_hi = x.view(np.uint8).reshape(B_FULL, D_IN, 4)[:, :, 3]
    w_hi = w.view(np.uint8).reshape(D_IN, UNITS, 4)[:, :, 3]
    # w: [d, u] -> [p, q, s, u']  with d = s*128 + p, u = q*512 + u'
    wt = w_hi.reshape(KT, P, NQ, 512).transpose(1, 2, 0, 3)
    w_core = np.ascontiguousarray(wt)
    in_maps = []
    for c in range(N_CORES):
        shard = x_hi[c * B_CORE:(c + 1) * B_CORE]          # [m, d]
        t = shard.T.reshape(KT, P, B_CORE).transpose(1, 0, 2)
        in_maps.append({
            "xhi": np.ascontiguousarray(t),                # [128,16,1024]
            "whi": w_core,
        })
    return in_maps


def kernel(x, w, _trace=False, _trace_cores=None):
    from concourse.bass_utils import run_bass_kernel_spmd

    x = np.asarray(x, dtype=np.float32)
    w = np.asarray(w, dtype=np.float32)
    assert x.shape == (B_FULL, D_IN) and w.shape == (D_IN, UNITS)

    nc = _get_nc()
    in_maps = _prep_inputs(x, w)
    br = run_bass_kernel_spmd(
        nc, in_maps, list(range(N_CORES)),
        trace=_trace, trace_cores=_trace_cores,
    )
    LAST_RESULTS["br"] = br
    out = np.concatenate(
        [br.results[c]["out"].astype(np.float32) for c in range(N_CORES)],
        axis=0,
    )
    return out


if __name__ == "__main__":
    rng = np.random.default_rng(0)
    x = rng.standard_normal((B_FULL, D_IN), dtype=np.float32)
    w = (rng.standard_normal((D_IN, UNITS), dtype=np.float32) * 0.1).astype(
        np.float32
    )
    out = kernel(x, w)
    exp = np.sign(x + (x == 0)) @ np.sign(w + (w == 0))
    print("max abs err:", np.max(np.abs(out - exp)))
